# revision 50
# baseline (speedup 1.0000x reference)
"""Trainium2 Bass kernel for causal self-attention (B=4, T=2048, C=1024, H=16).

Sharding: 8 cores = 4 batches x 2 head-groups (Megatron-style tensor parallel
over heads; the two head-group partial projection outputs are summed on host).

Per-core device pipeline (everything time-last, "transposed" layout):
  1. QKV projections (bf16 inputs, fp32 accumulate):
       QT, KT d-tiles [128=2 heads x 64, T] bf16;  V_aug t-tiles [128, 8*65] bf16
       (col h*65+64 = ones -> softmax denominator falls out of the AV matmul)
  2. Attention per (q-chunk, head-pair), causally restricted: for the diagonal
     key tiles only queries >= tile base are computed (S matmul N, exp free
     size, and AV matmul N all shrink).
       ST[tk, tq] = KT.T @ QT  (two row-packed K=64 matmuls into one psum)
       exp via ACT (scale=1/8, per-partition padding bias), bf16 out
       diag 128x128 sub-block: multiply by shared lower-tri 0/1 tile (DVE bf16)
       y[65, tq] += V_aug.T @ P  (bf16)
       divide: reciprocal of denom row, K=1 broadcast matmul, dual-PSUM DVE mul
  3. Output projection (bf16): o[cout, tq] = W_proj_g.T @ yn
Host: sum the two TP partials, transpose, add b_proj, zero padded rows.
"""

import numpy as np
import ml_dtypes
from contextlib import ExitStack

import concourse.bacc as bacc
import concourse.tile as tile
from concourse import mybir
from concourse.bass_utils import run_bass_kernel_spmd

dt = mybir.dt

N_HEAD = 16
C = 1024
HPG = 8          # heads per group (per core)
DH = 64          # head dim
NCH = C // 128   # contraction chunks for QKV
NEG = -1e9

_programs = {}


def build_program(T, has_bias=False):
    NQ = T // 512     # tq chunks
    NT = T // 128     # tk tiles
    nc = bacc.Bacc("TRN2", target_bir_lowering=False, debug=False)

    # host-prepped layouts: xt [128, 8, T] (c-chunk mid dim), weights packed
    # [128, chunks*cols] so each is a single DMA
    xt_d = nc.dram_tensor("xt", [128, NCH, T], dt.bfloat16, kind="ExternalInput")
    wq_d = nc.dram_tensor("wq", [128, NCH * 512], dt.bfloat16, kind="ExternalInput")
    wk_d = nc.dram_tensor("wk", [128, NCH * 512], dt.bfloat16, kind="ExternalInput")
    wv_d = nc.dram_tensor("wv", [128, NCH * 512], dt.bfloat16, kind="ExternalInput")
    wp_d = nc.dram_tensor("wp", [128, 4 * C], dt.bfloat16, kind="ExternalInput")
    bq_d = nc.dram_tensor("bq", [128, 4], dt.float32, kind="ExternalInput")
    bk_d = nc.dram_tensor("bk", [128, 4], dt.float32, kind="ExternalInput")
    bv_d = nc.dram_tensor("bv", [128, HPG * 65], dt.float32, kind="ExternalInput")
    pb_d = nc.dram_tensor("pb", [128, NT], dt.float32, kind="ExternalInput")
    tri_d = nc.dram_tensor("tri", [128, 256], dt.bfloat16, kind="ExternalInput")
    on_d = nc.dram_tensor("on", [65, 64], dt.float32r, kind="ExternalInput")
    o_d = nc.dram_tensor("o", [128, 8, T], dt.float32, kind="ExternalOutput")

    with tile.TileContext(nc) as tc, ExitStack() as ctx:
        # ---- persistent pools
        pool_const = ctx.enter_context(tc.tile_pool(name="const", bufs=1))
        pool_w = ctx.enter_context(tc.tile_pool(name="w", bufs=1))
        pool_qk = ctx.enter_context(tc.tile_pool(name="qk", bufs=1))
        pool_v = ctx.enter_context(tc.tile_pool(name="v", bufs=1))
        pool_xt = ctx.enter_context(tc.tile_pool(name="xt", bufs=2))
        pool_p = ctx.enter_context(tc.tile_pool(name="p", bufs=3))
        pool_yn = ctx.enter_context(tc.tile_pool(name="yn", bufs=2))
        pool_recip = ctx.enter_context(tc.tile_pool(name="recip", bufs=2))
        pool_o = ctx.enter_context(tc.tile_pool(name="o", bufs=2))
        # PSUM: s2 2x2 banks, y 2x1, shared qkv/proj/rb rotation 2x1 = 8 banks
        ps_s = ctx.enter_context(tc.tile_pool(name="ps_s", bufs=2, space="PSUM"))
        ps_y = ctx.enter_context(tc.tile_pool(name="ps_y", bufs=1, space="PSUM"))
        ps_q = ctx.enter_context(tc.tile_pool(name="ps_q", bufs=2, space="PSUM"))

        # ---- weights / persistent activations
        wq_t = pool_w.tile([128, NCH * 512], dt.bfloat16, name="wq")
        wk_t = pool_w.tile([128, NCH * 512], dt.bfloat16, name="wk")
        wv_t = pool_w.tile([128, NCH * 512], dt.bfloat16, name="wv")
        wp_t = pool_w.tile([128, 4 * C], dt.bfloat16, name="wp")

        qt_t = [pool_qk.tile([128, T], dt.bfloat16, tag=f"qt{d}", name=f"qt{d}") for d in range(4)]
        kt_t = [pool_qk.tile([128, T], dt.bfloat16, tag=f"kt{d}", name=f"kt{d}") for d in range(4)]
        v_t = [pool_v.tile([128, HPG * 65], dt.bfloat16, tag=f"v{t}", name=f"vt{t}") for t in range(NT)]

        xt_q = [None] * NQ
        # chunk-0 x and wv interleaved per c-slice so the first psv matmul
        # starts after ~256KB of DMA instead of 2MB
        xt_q[0] = pool_xt.tile([128, NCH * 512], dt.bfloat16, tag="x",
                               name="xt_c0")
        xt0_r = xt_q[0][:].rearrange("p (c t) -> p c t", c=NCH)
        for c in range(NCH):
            nc.sync.dma_start(xt0_r[:, c:c + 1, :], xt_d[:, c:c + 1, 0:512])
            nc.gpsimd.dma_start(wv_t[:, 512 * c:512 * c + 512],
                                wv_d[:, 512 * c:512 * c + 512])
        nc.scalar.dma_start(wq_t[:], wq_d[:])
        nc.scalar.dma_start(wk_t[:], wk_d[:])
        # small constants (not needed until the attention phase)
        bq_t = pool_const.tile([128, 4], dt.float32)
        nc.sync.dma_start(bq_t[:], bq_d[:])
        bk_t = pool_const.tile([128, 4], dt.float32)
        nc.sync.dma_start(bk_t[:], bk_d[:])
        bv_t = pool_const.tile([128, HPG * 65], dt.float32)
        nc.sync.dma_start(bv_t[:], bv_d[:])
        pb_t = pool_const.tile([128, NT], dt.float32)
        nc.sync.dma_start(pb_t[:], pb_d[:])
        tri_t = pool_const.tile([128, 256], dt.bfloat16)
        nc.sync.dma_start(tri_t[:], tri_d[:])
        ones_t = pool_const.tile([65, DH], dt.float32r)
        nc.sync.dma_start(ones_t[:], on_d[:])
        nc.scalar.dma_start(wp_t[:], wp_d[:])
        if not has_bias:
            # ones column of V_aug written once (persistent tiles)
            for t in range(NT):
                nc.vector.memset(
                    v_t[t][:].rearrange("p (h e) -> p h e", h=HPG)[:, :, DH:DH + 1],
                    1.0)

        def do_qkv(q):
            tq = slice(512 * q, 512 * q + 512)
            xt_c = xt_q[q]
            # V tiles (natural [t, d] layout, strided V_aug with ones cols)
            for tl in range(4):
                psv = ps_q.tile([128, 512], dt.float32, tag="q",
                                name=f"psv{q}_{tl}")
                for c in range(NCH):
                    nc.tensor.matmul(psv[:],
                                     xt_c[:, 512 * c + 128 * tl:512 * c + 128 * tl + 128],
                                     wv_t[:, 512 * c:512 * c + 512],
                                     start=(c == 0), stop=(c == NCH - 1))
                vt = v_t[4 * q + tl]
                vt_r = vt[:].rearrange("p (h e) -> p h e", h=HPG)
                if has_bias:
                    nc.vector.tensor_scalar_add(
                        vt_r[:, :, 0:DH],
                        psv[:].rearrange("p (h e) -> p h e", h=HPG),
                        0.0)
                    nc.vector.tensor_add(
                        vt_r[:, :, 0:DH], vt_r[:, :, 0:DH],
                        bv_t[:].rearrange("p (h e) -> p h e", h=HPG)[:, :, 0:DH])
                    nc.vector.tensor_copy(
                        vt_r[:, :, DH:DH + 1],
                        bv_t[:].rearrange("p (h e) -> p h e", h=HPG)[:, :, DH:DH + 1])
                else:
                    nc.vector.tensor_copy(
                        vt_r[:, :, 0:DH],
                        psv[:].rearrange("p (h e) -> p h e", h=HPG))
            # Q and K (transposed [d, t] layout)
            for mi, (w_t, dst, bias_t) in enumerate(
                    ((wq_t, qt_t, bq_t), (wk_t, kt_t, bk_t))):
                for d in range(4):
                    psq = ps_q.tile([128, 512], dt.float32, tag="q",
                                    name=f"psq{q}_{mi}_{d}")
                    for c in range(NCH):
                        nc.tensor.matmul(psq[:],
                                         w_t[:, 512 * c + 128 * d:512 * c + 128 * d + 128],
                                         xt_c[:, 512 * c:512 * c + 512],
                                         start=(c == 0), stop=(c == NCH - 1))
                    if has_bias:
                        nc.vector.tensor_scalar_add(dst[d][:, tq], psq[:],
                                                    bias_t[:, d:d + 1])
                    else:
                        nc.vector.tensor_copy(dst[d][:, tq], psq[:])

        def do_attn(q):
            tq = slice(512 * q, 512 * q + 512)
            yn_q = [pool_yn.tile([128, 512], dt.bfloat16, tag=f"yn{d}", name=f"yn{d}_{q}")
                    for d in range(4)]
            n_tk = 4 * q + 4
            for hp in range(4):
                y_ps = [ps_y.tile([65, 512], dt.float32, tag=f"y{h}",
                                  name=f"y{h}_{q}_{hp}") for h in range(2)]

                def do_av(t_idx, p2):
                    v = t_idx - 4 * q
                    lo = 128 * v if v >= 0 else 0
                    for h in range(2):
                        va = v_t[t_idx][:, 65 * (2 * hp + h):65 * (2 * hp + h) + 65]
                        nc.tensor.matmul(y_ps[h][:, lo:512], va,
                                         p2[:, 512 * h + lo:512 * h + 512],
                                         start=(t_idx == 0),
                                         stop=(t_idx == n_tk - 1))

                prev = None  # (t_idx, p2): AV lags S by one tile so PE never
                #              waits on the exp of the tile it just produced
                for t_idx in range(n_tk):
                    v = t_idx - 4 * q  # diag sub-tile index if >= 0
                    lo = 128 * v if v >= 0 else 0
                    tk = slice(128 * t_idx, 128 * t_idx + 128)
                    tqr = slice(512 * q + lo, 512 * q + 512)
                    s2 = ps_s.tile([128, 1024], dt.float32, tag="s")
                    nc.tensor.matmul(s2[:, lo:512], kt_t[hp][0:64, tk],
                                     qt_t[hp][0:64, tqr], start=True, stop=True,
                                     tile_position=(0, 0))
                    nc.tensor.matmul(s2[:, 512 + lo:1024], kt_t[hp][64:128, tk],
                                     qt_t[hp][64:128, tqr], start=True, stop=True,
                                     tile_position=(64, 0))
                    p2 = pool_p.tile([128, 1024], dt.bfloat16, tag="p")
                    s2_r = s2[:].rearrange("p (s n) -> p s n", s=2)
                    p2_r = p2[:].rearrange("p (s n) -> p s n", s=2)
                    nc.scalar.activation(p2_r[:, :, lo:512], s2_r[:, :, lo:512],
                                         mybir.ActivationFunctionType.Exp,
                                         bias=pb_t[:, t_idx:t_idx + 1], scale=0.125)
                    if v >= 0:  # diagonal: mask the [128,128] tri sub-block
                        nc.vector.tensor_mul(
                            p2_r[:, :, lo:lo + 128], p2_r[:, :, lo:lo + 128],
                            tri_t[:].rearrange("p (s n) -> p s n", s=2))
                    if prev is not None:
                        do_av(*prev)
                    prev = (t_idx, p2)
                do_av(*prev)
                for h in range(2):
                    recip = pool_recip.tile([65, 512], dt.float32r, tag="r")
                    with nc.allow_low_precision(reason="f32r is 32-bit"):
                        nc.vector.reciprocal(recip[64:65, :], y_ps[h][64:65, :])
                    rb = ps_q.tile([64, 512], dt.float32, tag="q",
                                   name=f"rb{q}_{hp}_{h}")
                    nc.tensor.matmul(rb[:], ones_t[64:65, :], recip[64:65, :],
                                     start=True, stop=True)
                    rb_sb = pool_recip.tile([64, 512], dt.float32r, tag="rb")
                    nc.vector.tensor_copy(rb_sb[:], rb[:])
                    with nc.allow_low_precision(reason="probs in [0,1]"):
                        nc.vector.tensor_mul(yn_q[hp][64 * h:64 * h + 64, :],
                                             y_ps[h][0:64, :], rb_sb[:])
            return yn_q

        def do_proj(q, yn_q):
            tq = slice(512 * q, 512 * q + 512)
            o_sb = pool_o.tile([128, 8 * 512], dt.float32, tag="o",
                               name=f"osb{q}")
            for ct in range(8):
                pso = ps_q.tile([128, 512], dt.float32, tag="q",
                                name=f"pso{q}_{ct}")
                for d in range(4):
                    nc.tensor.matmul(pso[:],
                                     wp_t[:, C * d + 128 * ct:C * d + 128 * ct + 128],
                                     yn_q[d][:], start=(d == 0), stop=(d == 3))
                nc.vector.tensor_copy(o_sb[:, 512 * ct:512 * ct + 512], pso[:])
            nc.gpsimd.dma_start(
                o_d[:, :, tq],
                o_sb[:].rearrange("p (c t) -> p c t", c=8))

        # software pipeline: QKV(q+1) is emitted before proj(q) so PE has
        # work while the last head-pair's divide chain completes
        do_qkv(0)
        for q in range(NQ):
            if q + 1 < NQ:
                tq1 = slice(512 * (q + 1), 512 * (q + 1) + 512)
                xt_q[q + 1] = pool_xt.tile([128, NCH * 512], dt.bfloat16,
                                           tag="x", name=f"xt_c{q + 1}")
                nc.sync.dma_start(
                    xt_q[q + 1][:].rearrange("p (c t) -> p c t", c=NCH),
                    xt_d[:, :, tq1])
            yn_q = do_attn(q)
            if q + 1 < NQ:
                do_qkv(q + 1)
            do_proj(q, yn_q)

    nc.compile()
    return nc


def get_program(T, has_bias=False):
    key = (T, has_bias)
    if key not in _programs:
        _programs[key] = build_program(T, has_bias)
    return _programs[key]


def make_core_inputs(x, padding_mask, W_attn, b_attn, W_proj, b_proj, core):
    B, T, Cx = x.shape
    b, g = core // 2, core % 2
    bf16 = ml_dtypes.bfloat16
    cs = slice(512 * g, 512 * g + 512)

    # xt [128, NCH, T]: xt[p, c, t] = x[b, t, 128c+p]
    xt = np.ascontiguousarray(
        x[b].T.reshape(NCH, 128, T).transpose(1, 0, 2)).astype(bf16)

    def pack_w(w):  # [1024, 512] -> [128, NCH*512]
        return np.ascontiguousarray(
            w.reshape(NCH, 128, 512).transpose(1, 0, 2)
            .reshape(128, NCH * 512)).astype(bf16)

    wq = pack_w(W_attn[:, cs])
    wk = pack_w(W_attn[:, 1024 + 512 * g:1024 + 512 * g + 512])
    wv = pack_w(W_attn[:, 2048 + 512 * g:2048 + 512 * g + 512])
    # wp [128, 4*1024]: wp[p, 1024d+j] = W_proj[cs][128d+p, j]
    wp = np.ascontiguousarray(
        W_proj[cs, :].reshape(4, 128, C).transpose(1, 0, 2).reshape(128, 4 * C)
    ).astype(bf16)
    bq = np.ascontiguousarray(b_attn[cs].reshape(4, 128).T).astype(np.float32)
    bk = np.ascontiguousarray(
        b_attn[1024 + 512 * g:1024 + 512 * g + 512].reshape(4, 128).T
    ).astype(np.float32)
    bvv = b_attn[2048 + 512 * g:2048 + 512 * g + 512].astype(np.float32)
    bv_row = np.zeros(HPG * 65, np.float32)
    for h in range(HPG):
        bv_row[65 * h:65 * h + 64] = bvv[64 * h:64 * h + 64]
        bv_row[65 * h + 64] = 1.0
    bv = np.ascontiguousarray(np.broadcast_to(bv_row, (128, HPG * 65))).astype(np.float32)
    NT = T // 128
    pb = np.where(padding_mask[b].reshape(NT, 128).T, np.float32(NEG), np.float32(0.0))
    pb = np.ascontiguousarray(pb).astype(np.float32)
    # shared lower-tri 0/1 tile for diagonal sub-blocks, twice side by side
    p_idx = np.arange(128)[:, None]
    j_idx = np.arange(128)[None, :]
    tri1 = np.where(p_idx <= j_idx, np.float32(1.0), np.float32(0.0))
    tri = np.ascontiguousarray(np.tile(tri1, (1, 2))).astype(bf16)
    return {"xt": xt, "wq": wq, "wk": wk, "wv": wv, "wp": wp,
            "bq": bq, "bk": bk, "bv": bv, "pb": pb, "tri": tri,
            "on": np.ones((65, 64), np.float32)}


def combine_outputs(results, x, padding_mask, b_proj):
    B, T, Cx = x.shape
    out = np.empty((B, T, Cx), np.float32)
    for b in range(B):
        # o [128, 8, T] -> [C, T]
        o0 = results[2 * b]["o"].transpose(1, 0, 2).reshape(Cx, T)
        o1 = results[2 * b + 1]["o"].transpose(1, 0, 2).reshape(Cx, T)
        y = (o0 + o1).T + b_proj[None, :]
        y[padding_mask[b]] = 0.0
        out[b] = y
    return out


def kernel(x, padding_mask, W_attn, b_attn, W_proj, b_proj):
    x = np.asarray(x)
    padding_mask = np.asarray(padding_mask)
    W_attn = np.asarray(W_attn, np.float32)
    b_attn = np.asarray(b_attn, np.float32)
    W_proj = np.asarray(W_proj, np.float32)
    b_proj = np.asarray(b_proj, np.float32)
    B, T, Cx = x.shape
    has_bias = bool(np.any(b_attn != 0))
    nc = get_program(T, has_bias)
    in_maps = [make_core_inputs(x, padding_mask, W_attn, b_attn, W_proj, b_proj, core)
               for core in range(8)]
    res = run_bass_kernel_spmd(nc, in_maps, list(range(8)))
    return combine_outputs(res.results, x, padding_mask, b_proj)
